# revision 21
# baseline (speedup 1.0000x reference)
"""Trainium2 Bass kernel for a single attention head with query-axis softmax.

Reference semantics (per batch b):
    k = x @ Wk; q = x @ Wq; v = x @ Wv                 # [T, H]
    wei = (q @ k^T) * E**-0.5                          # [T(query), T(key)]
    wei = where(tril, wei, -inf)                       # causal: keep s <= t
    p = softmax(wei, axis=0 over query t)              # NOTE: query axis!
    out = p @ v                                        # [T, H]

Because the softmax normalizes over the query axis t (per key column s),
out[t,h] = sum_s E[t,s] * v[s,h] / d[s] with E[t,s] = exp(wei[t,s])
(zero for s > t) and d[s] = sum_t E[t,s].  The kernel computes E^T tiles
([s on partitions, t free]) so d is a free-axis row sum (fused into the
exp instruction via accum_out), scales v rows by 1/d, and accumulates
out^T on PE.  out^T is stored as-is; the host un-transposes during the
gather (free), so no on-device layout fixup is needed.

Key performance structure (v2):
  * The S matmuls contract over only H=64, so they run as TWO CONCURRENT
    K=64 row-group tiles on the PE (tile_position row packing): group A
    uses array rows 0:63 (stationary kT at partitions 0:64, moving q at
    partitions 0:64), group B rows 64:127 (both at base 64).  Adjacent
    A/B matmuls overlap nearly fully -> ~2x S throughput vs the padded
    K=128 form.  This requires kT and q each to exist in BOTH partition
    halves:
      - kT is duplicated for free by projecting with the stationary
        [Wk_e | Wk_e] (output partitions 0:64 and 64:128 both get kT).
      - q is projected at partitions 0:64 ([Wq_e | Wv_e] chain) and
        copied to partitions 64:128 with a small SBUF->SBUF DMA.
  * The additive -1e30 causal triangle for the diagonal block is split
    into two concurrent K=64 matmuls (identity top/bottom halves against
    mask rows 0:64 / 64:128).
  * v^T (at partitions 64:128 from the [Wq|Wv] chain) is re-transposed
    to natural [s, h] with K=64 group-B matmuls against the identity's
    bottom-right quadrant, overlapping group-A S work.
  * exp runs one scalar-engine activation per <=1024-col row piece, each
    fusing the d partial sum via accum_out.  PSUM budget: 2 banks out^T
    accumulators + 2 banks projection/v-transpose ring + 4 banks
    (2 x [128,1024]) double-buffered S row pieces = 8.
  * AV matmuls (M=64) pair automatically via output base-partition
    alternation (col_grp h0/h64) and lag the exp stream so the PE never
    waits on the d / v' chain.

Sharding: batch dim (8) across the 8 NeuronCores, weights replicated.
x is host-packed per column block ([NJ, 128, NE*CB] bf16).  DMA issues
cost ~0.65us each and only sync/scalar/gpsimd queues can issue, so the
front splits block 3 + the weight chunks into ~256KB pieces across the
three queues (projection drips as pieces land), and later blocks are
paced naturally: their issues sit on the sync queue behind the q-dup
DMAs (whose data waits pace them) and on the gpsimd queue behind small
WAW fences keyed to the previous block's projection cast.  A short
junk-matmul warmup keeps the PE activity monitor from holding the clock
at half rate during the initial DMA fill.
"""

import os

BISECT = os.environ.get("BISECT", "")

import numpy as np
import ml_dtypes

import concourse.bass as bass
import concourse.tile as tile
from concourse import bacc, mybir
from concourse import bass_utils

B, T, E, H = 8, 2048, 1024, 64
P = 128                       # partitions
CB = 512                      # column block (t) width
NE = E // P                   # 8 contraction chunks for projections
NJ = T // CB                  # 4 column blocks
SCALE = float(E) ** -0.5      # note: embed**-0.5, not head_size**-0.5
MASK_NEG = -1.0e30
F32 = mybir.dt.float32
BF16 = mybir.dt.bfloat16
EXP = mybir.ActivationFunctionType.Exp
COPY = mybir.ActivationFunctionType.Copy

# packed weights tensor column offsets (all bf16).
# chunk pair e: [kk_e (128) | qv_e (128)] so one stream drips pairs in
# consumption order. kk_e = [Wk_e | Wk_e], qv_e = [Wq_e | Wv_e].
KKQV0 = 0                     # [P, NE*2*P]
MASK0 = KKQV0 + NE * 2 * P    # [P, 4*P]    additive -1e30 triangles
ID0 = MASK0 + 4 * P           # [P, P]      identity (I128)
WALLW = ID0 + P


def _emit(tc, xb_d, wall_d, out_d):
    nc = tc.nc
    from contextlib import ExitStack

    with ExitStack() as ctx:
        singles = ctx.enter_context(tc.tile_pool(name="singles", bufs=1))
        epool = ctx.enter_context(tc.tile_pool(name="erow", bufs=9))
        dpool = ctx.enter_context(tc.tile_pool(name="dsmall", bufs=12))
        vpool = ctx.enter_context(tc.tile_pool(name="vrow", bufs=9))
        vspool = ctx.enter_context(tc.tile_pool(name="vstage", bufs=2))
        # S row pieces: [128, 1024] f32 = 2 PSUM banks each, ring of 2
        spool = ctx.enter_context(tc.tile_pool(name="sp", bufs=2, space="PSUM"))
        # projection chains + v-transpose staging: 1 bank each, ring of 2
        ppool = ctx.enter_context(tc.tile_pool(name="pp", bufs=2, space="PSUM"))
        pout = ctx.enter_context(tc.tile_pool(name="pout", bufs=1, space="PSUM"))

        # --- weights tiles -------------------------------------------------
        wall = singles.tile([P, MASK0], BF16, name="wall")
        wall2 = singles.tile([P, WALLW - MASK0], BF16, name="wall2")
        masks = wall2[:, 0 : 4 * P]
        identb = wall2[:, 4 * P : 5 * P]

        def kk_sl(e):
            return wall[:, 2 * P * e : 2 * P * e + P]

        def qv_sl(e):
            return wall[:, 2 * P * e + P : 2 * P * e + 2 * P]

        # --- x blocks: lo/hi halves, 4KB/partition per transfer (2KB runs
        # halve the effective DMA rate) ------------------------------------
        HALF = NE * CB // 2
        xlo = {j: singles.tile([P, HALF], BF16, name=f"xlo{j}") for j in range(NJ)}
        xhi = {j: singles.tile([P, HALF], BF16, name=f"xhi{j}") for j in range(NJ)}

        # warmup operand (values never affect results)
        junk = nc.alloc_sbuf_tensor("junk_warm", [P, 4 * P], BF16).ap()

        # front: transfers split by PARTITION halves (full-length 4KB runs;
        # column splits degrade to 2KB runs and halve the DMA rate).  x3
        # rides four streams, weights two, all concurrent from t~0; x2 is
        # paced behind x3's completion via gpsimd WAW fences.
        nc.gpsimd.memset(junk[:, 0 : 2 * P], 1.0)
        nc.sync.dma_start(out=wall[0:H, :], in_=wall_d[0:H, 0:MASK0])
        nc.gpsimd.dma_start(out=xlo[3][0:H, :], in_=xb_d[3][0:H, 0:HALF])
        nc.scalar.dma_start(out=xhi[3][0:H, :], in_=xb_d[3][0:H, HALF:])
        nc.sync.dma_start(out=wall[H:P, :], in_=wall_d[H:P, 0:MASK0])
        nc.gpsimd.dma_start(out=xlo[3][H:P, :], in_=xb_d[3][H:P, 0:HALF])
        nc.sync.dma_start(out=xhi[3][H:P, :], in_=xb_d[3][H:P, HALF:])
        nc.scalar.dma_start(out=wall2[:], in_=wall_d[:, MASK0:])
        # x2 paced behind x3 so block 3 owns the wire first
        nc.gpsimd.tensor_copy(xlo[2][0:1, 0:1], xlo[3][0:1, 0:1])
        nc.gpsimd.tensor_copy(xlo[2][H : H + 1, 0:1], xlo[3][H : H + 1, 0:1])
        nc.gpsimd.dma_start(out=xlo[2][:], in_=xb_d[2][:, 0:HALF])
        nc.gpsimd.tensor_copy(xhi[2][0:1, 0:1], xhi[3][0:1, 0:1])
        nc.gpsimd.tensor_copy(xhi[2][H : H + 1, 0:1], xhi[3][H : H + 1, 0:1])
        nc.scalar.dma_start(out=xhi[2][:], in_=xb_d[2][:, HALF:])

        def x_rhs(j, e):
            t_ = xlo[j] if e < NE // 2 else xhi[j]
            c = (e % (NE // 2)) * CB
            return t_[:, c : c + CB]

        # --- warmup: keep the PE busy from the start so the HAM clock gate
        # opens before the first projection; junk values are never read.
        pwarm = spool.tile([P, 2 * CB], F32, tag="sp", name="pwarm")
        NWARM = 16
        for w in range(NWARM):
            nc.tensor.matmul(
                pwarm[:, 0:P],
                lhsT=junk[:, 0:P],
                rhs=junk[:, P : 2 * P],
                start=True,
                stop=True,
            )

        # --- per-block SBUF results ---------------------------------------
        kdup = {j: singles.tile([P, CB], BF16, name=f"kdup{j}") for j in range(NJ)}
        vtmp = {j: singles.tile([P, CB], BF16, name=f"vtmp{j}") for j in range(NJ)}
        # q2: rows 0:64 = q (from projection), rows 64:128 = DMA-shifted dup
        q2 = singles.tile([P, T], BF16, name="q2")

        # out^T accumulators packed 2 per bank: jj even rows 0:64, odd 64:128.
        pout_tiles = [
            pout.tile([P, CB], F32, tag=f"pt{a}", name=f"pt{a}") for a in range(2)
        ]
        outst = singles.tile([P, 2 * CB], BF16, name="outst")

        def pout_slice(jj, c0, c1):
            rb = H * (jj % 2)
            return pout_tiles[jj // 2][rb : rb + H, c0:c1]

        # deferred AV emission (lag behind S so PE never waits on the
        # d / v' chain): each entry = (r, j_of_row), d0, erow, vi
        pending_av = []

        def _av_one(rj, d0, erow, vi, jj):
            c = (jj - rj[1]) * CB
            lo = d0 if jj == rj[1] else 0
            nc.tensor.matmul(
                pout_slice(jj, lo, CB),
                lhsT=vi[:],
                rhs=erow[:, c + lo : c + CB],
                start=(jj == rj[1] and rj[0] == 0),
                stop=(rj[1] == 0 and rj[0] == 3),
                skip_group_check=True,
            )

        def close_bank(a):
            # stage out^T bank a PSUM->SBUF, split across vector and scalar
            # so the copy's latency halves, then store it
            half = outst[:, a * CB : (a + 1) * CB]
            nc.vector.tensor_copy(half[:, 0 : CB // 2], pout_tiles[a][:, 0 : CB // 2])
            nc.scalar.activation(
                out=half[:, CB // 2 : CB],
                in_=pout_tiles[a][:, CB // 2 : CB],
                func=COPY,
            )
            eng0 = nc.sync if a == 0 else nc.scalar
            eng0.dma_start(
                out=out_d[:, a * CB : a * CB + CB // 2],
                in_=half[:, 0 : CB // 2],
            )
            nc.gpsimd.dma_start(
                out=out_d[:, a * CB + CB // 2 : (a + 1) * CB],
                in_=half[:, CB // 2 :],
            )

        def flush_av(final):
            if final:
                rows = list(pending_av)
                pending_av.clear()
                for rj, d0, erow, vi in rows[:-1]:
                    for jj in range(rj[1], NJ):
                        _av_one(rj, d0, erow, vi, jj)
                rj, d0, erow, vi = rows[-1]
                for jj in range(rj[1], 2):
                    _av_one(rj, d0, erow, vi, jj)
                close_bank(0)
                for jj in range(2, NJ):
                    _av_one(rj, d0, erow, vi, jj)
                close_bank(1)
                return
            rj, d0, erow, vi = pending_av.pop(0)
            for jj in range(rj[1], NJ):
                _av_one(rj, d0, erow, vi, jj)

        # --- projections ---------------------------------------------------
        def proj_thunks(j):
            pkk = ppool.tile([P, CB], F32, tag="pp", name=f"pkk{j}")
            pqv = ppool.tile([P, CB], F32, tag="pp", name=f"pqv{j}")

            def kk_mm(e):
                nc.tensor.matmul(
                    pkk[:],
                    lhsT=kk_sl(e),
                    rhs=x_rhs(j, e),
                    start=(e == 0),
                    stop=(e == NE - 1),
                )

            def qv_mm(e):
                nc.tensor.matmul(
                    pqv[:],
                    lhsT=qv_sl(e),
                    rhs=x_rhs(j, e),
                    start=(e == 0),
                    stop=(e == NE - 1),
                )

            thunks = []
            for e in range(NE):
                thunks.append(lambda e=e: kk_mm(e))
                thunks.append(lambda e=e: qv_mm(e))
            return (pkk, pqv), thunks

        def proj_cast(j, pkq):
            pkk, pqv = pkq
            # q cast first so the q-dup DMA (and with it the diagonal S
            # matmul's full-rhs dependency) starts as early as possible
            nc.vector.tensor_copy(q2[0:H, j * CB : (j + 1) * CB], pqv[0:H, :])
            nc.vector.tensor_copy(kdup[j][:], pkk[:])
            if BISECT not in ("B", "Bvt"):
                nc.vector.tensor_copy(vtmp[j][:], pqv[:])
            else:
                nc.vector.tensor_copy(vtmp[j][H:P, :], pqv[H:P, :])

        def v_transpose(j):
            # group-B (array rows 64:127) K=64 matmuls against the identity
            # bottom-right quadrant: vps[s, h] = vT[h, s]
            vps = ppool.tile([P, 4 * H], F32, tag="pp", name=f"vps{j}")
            vstage = vspool.tile([P, 4 * H], BF16, name=f"vstage{j}")
            for rr in range(4):
                if BISECT not in ("B", "Bvt"):
                    nc.tensor.matmul(
                        vps[:, rr * H : (rr + 1) * H],
                        lhsT=vtmp[j][:, rr * P : (rr + 1) * P],
                        rhs=identb[:, H:P],
                        start=True,
                        stop=True,
                    )
                else:
                    nc.tensor.matmul(
                        vps[:, rr * H : (rr + 1) * H],
                        lhsT=vtmp[j][H:P, rr * P : (rr + 1) * P],
                        rhs=identb[H:P, H:P],
                        start=True,
                        stop=True,
                    )
            nc.vector.tensor_copy(vstage[:], vps[:])
            return vstage

        # --- main pipeline: column blocks in descending order --------------
        bctr = [0]
        next_proj = []

        def drip_proj(k):
            for _ in range(min(k, len(next_proj))):
                next_proj.pop(0)()

        pkq, thunks = proj_thunks(3)
        for t_ in thunks:
            t_()
        proj_cast(3, pkq)
        # x1 paced behind block 3's projection cast (gpsimd WAW fences; the
        # sync issue rides the same tile dependency)
        nc.gpsimd.tensor_copy(xlo[1][0:1, 0:1], kdup[3][0:1, 0:1])
        nc.sync.dma_start(out=xlo[1][:], in_=xb_d[1][:, 0:HALF])
        nc.gpsimd.tensor_copy(xhi[1][0:1, 0:1], kdup[3][0:1, 0:1])
        nc.gpsimd.dma_start(out=xhi[1][:], in_=xb_d[1][:, HALF:])

        vstage = None
        for j in reversed(range(NJ)):
            nblk = NJ - j
            npiece = (nblk + 1) // 2

            for r in range(4):
                d0 = r * P
                t0 = j * CB
                erow = epool.tile([P, T], BF16)
                dparts = dpool.tile([P, 2], F32, tag="dparts")
                kA = kdup[j][0:H, d0 : d0 + P]
                kB = kdup[j][H:P, d0 : d0 + P]

                pieces = []
                for pc in range(npiece):
                    w = min(2 * CB, nblk * CB - 2 * CB * pc)
                    pst_t = spool.tile([P, 2 * CB], F32, tag="sp", name="pst")
                    pieces.append((pst_t, w, 2 * CB * pc))

                # diagonal 128-block: concurrent mask halves, then S diag
                p0 = pieces[0][0]
                rm = slice(r * P, (r + 1) * P)
                use_split_mask = BISECT in ("B", "Bmask")
                if not use_split_mask:
                    nc.tensor.matmul(
                        p0[:, d0 : d0 + P],
                        lhsT=identb[:],
                        rhs=masks[:, rm],
                        start=True,
                        stop=False,
                    )
                else:
                    nc.tensor.matmul(
                        p0[:, d0 : d0 + P],
                        lhsT=identb[0:H, :],
                        rhs=masks[0:H, rm],
                        start=True,
                        stop=False,
                    )
                    nc.tensor.matmul(
                        p0[:, d0 : d0 + P],
                        lhsT=identb[H:P, :],
                        rhs=masks[H:P, rm],
                        start=False,
                        stop=False,
                    )
                nc.tensor.matmul(
                    p0[:, d0 : d0 + P],
                    lhsT=kA,
                    rhs=q2[0:H, t0 + d0 : t0 + d0 + P],
                    start=False,
                    stop=True,
                )

                # remaining S columns in <=512 pieces, alternating row-group
                # A/B so adjacent matmuls run concurrently on the PE
                grp = 1
                c = d0 + P
                wtot = nblk * CB
                while c < wtot:
                    ce = min(c + CB - c % CB, wtot)
                    pc = c // (2 * CB)
                    pst, _, cofs = pieces[pc]
                    if BISECT in ("B", "BS"):
                        kx = kB if grp else kA
                        qx = (
                            q2[H:P, t0 + c : t0 + ce]
                            if grp
                            else q2[0:H, t0 + c : t0 + ce]
                        )
                        nc.tensor.matmul(
                            pst[:, c - cofs : ce - cofs],
                            lhsT=kx,
                            rhs=qx,
                            start=True,
                            stop=True,
                        )
                    else:
                        nc.tensor.matmul(
                            pst[:, c - cofs : ce - cofs],
                            lhsT=kA,
                            rhs=q2[0:H, t0 + c : t0 + ce],
                            start=True,
                            stop=True,
                        )
                    grp ^= 1
                    c = ce

                if r == 0:
                    # block j's v' transposes ride behind row 0's S matmuls
                    # (group B, overlapping); needed only from row 0's vi
                    vstage = v_transpose(j)
                    if j > 0:
                        pkq_next, next_proj = proj_thunks(j - 1)
                        if j == 2:
                            # x0 paced behind block 2's projection cast
                            nc.gpsimd.tensor_copy(
                                xlo[0][0:1, 0:1], kdup[2][0:1, 0:1]
                            )
                            nc.sync.dma_start(
                                out=xlo[0][:], in_=xb_d[0][:, 0:HALF]
                            )
                            nc.gpsimd.tensor_copy(
                                xhi[0][0:1, 0:1], kdup[2][0:1, 0:1]
                            )
                            nc.gpsimd.dma_start(
                                out=xhi[0][:], in_=xb_d[0][:, HALF:]
                            )

                drip_proj(4)

                # exp (+ d partial sums fused via accum_out), per piece
                exp_scale = SCALE * 0.5 if False else SCALE
                for pc, (pst, w, cofs) in enumerate(pieces):
                    lo = d0 if pc == 0 else 0
                    nc.scalar.activation(
                        out=erow[:, cofs + lo : cofs + w],
                        in_=pst[:, lo:w],
                        func=EXP,
                        scale=exp_scale,
                        accum_out=dparts[:, pc : pc + 1],
                    )

                dinv = dpool.tile([P, 1], F32, tag="dinv")
                if npiece > 1:
                    dsum = dpool.tile([P, 1], F32, tag="dsum")
                    nc.vector.tensor_add(dsum[:], dparts[:, 0:1], dparts[:, 1:2])
                    nc.vector.reciprocal(dinv[:], dsum[:])
                else:
                    nc.vector.reciprocal(dinv[:], dparts[:, 0:1])

                vi = vpool.tile([P, H], BF16, tag="vi", name="vi")
                nc.vector.tensor_scalar_mul(
                    vi[:], vstage[:, r * H : (r + 1) * H], dinv[:]
                )

                lag = 5 if j == 1 else 2
                if len(pending_av) >= lag:
                    flush_av(False)
                drip_proj(2)
                pending_av.append(((r, j), d0, erow, vi))

            # drain remaining next-step projection matmuls, then its cast
            drip_proj(len(next_proj))
            if j > 0:
                proj_cast(j - 1, pkq_next)

        flush_av(True)


def _enable_ldw_opt():
    """Flip walrus's --enable-ldw-opt to true for our compile: consecutive
    matmuls reusing the same stationary operand then skip the reload."""
    import concourse.bass_utils as bu

    if getattr(bu, "_ldw_opt_patched", False):
        return
    orig = bu.run_command

    def run_command_ldw(cmd, *a, **kw):
        if isinstance(cmd, list):
            cmd = [
                "--enable-ldw-opt=true" if c == "--enable-ldw-opt=false" else c
                for c in cmd
            ]
        return orig(cmd, *a, **kw)

    bu.run_command = run_command_ldw
    bu._ldw_opt_patched = True


def _build_program():
    if os.environ.get("BASS_LDW_OPT", "0") == "1":
        _enable_ldw_opt()
    nc = bacc.Bacc("TRN2", target_bir_lowering=False, debug=False, num_devices=B)
    xb_d = nc.dram_tensor("xb", [NJ, P, NE * CB], BF16, kind="ExternalInput").ap()
    wall_d = nc.dram_tensor("wall", [P, WALLW], BF16, kind="ExternalInput").ap()
    out_d = nc.dram_tensor("out", [P, 2 * CB], BF16, kind="ExternalOutput").ap()
    with tile.TileContext(nc) as tc:
        _emit(tc, xb_d, wall_d, out_d)
    nc.compile()
    return nc


def _host_masks():
    """[128, 4*128] additive triangles: row r masks t < s within the
    diagonal 128-block (t-local f, partition p: keep f >= p)."""
    m = np.full((P, 4 * P), MASK_NEG, dtype=np.float32)
    p = np.arange(P)[:, None]
    f = np.arange(P)[None, :]
    for r in range(4):
        m[:, r * P : (r + 1) * P][f >= p] = 0.0
    return m


def _host_inputs(x, Wk, Wq, Wv):
    bf = ml_dtypes.bfloat16
    x = np.asarray(x, dtype=np.float32)
    # [B, E, T] -> block-major [B, NJ, P, NE*CB] so each block is one
    # contiguous DMA with 4KB/partition runs
    xT = np.transpose(x, (0, 2, 1)).reshape(B, NE, P, NJ, CB)
    xb = np.ascontiguousarray(xT.transpose(0, 3, 2, 1, 4)).reshape(
        B, NJ, P, NE * CB
    ).astype(bf)

    def chunks(w):  # [E, h] -> [NE, P, h]
        return np.asarray(w, np.float32).reshape(NE, P, -1)

    # chunk pair e: [Wk_e | Wk_e | Wq_e | Wv_e] -> [P, NE*256]
    kc, qc, vc = chunks(Wk), chunks(Wq), chunks(Wv)
    kkqv = np.concatenate([kc, kc, qc, vc], axis=2)  # [NE, P, 256]
    kkqv = kkqv.transpose(1, 0, 2).reshape(P, NE * 2 * P)
    wallnp = np.concatenate(
        [kkqv, _host_masks(), np.eye(P, dtype=np.float32)], axis=1
    ).astype(bf)
    assert wallnp.shape == (P, WALLW)
    return [{"xb": xb[b], "wall": wallnp} for b in range(B)]


def _unpack_out(outT):
    """[128, 1024] out^T banks (bf16) -> [T, H] f32 natural layout."""
    outT = np.asarray(outT, dtype=np.float32)
    o = np.empty((T, H), dtype=np.float32)
    for a in range(2):
        for h2 in range(2):
            jj = 2 * a + h2
            o[jj * CB : (jj + 1) * CB, :] = outT[
                H * h2 : H * (h2 + 1), a * CB : (a + 1) * CB
            ].T
    return o


def _ensure_axon_ntff_hook():
    """The agent image's antenv lacks axon_hooks; synthesize it so
    run_bass_kernel_spmd's trace path can find the NTFF profile hook."""
    import sys
    import types

    if "antenv.axon_hooks" in sys.modules:
        return
    try:
        import antenv

        mod = types.ModuleType("antenv.axon_hooks")
        mod._hook = None

        def set_axon_ntff_profile_hook(h):
            mod._hook = h

        def get_axon_ntff_profile_hook():
            return mod._hook

        mod.set_axon_ntff_profile_hook = set_axon_ntff_profile_hook
        mod.get_axon_ntff_profile_hook = get_axon_ntff_profile_hook
        sys.modules["antenv.axon_hooks"] = mod
        antenv.axon_hooks = mod

        from trn_agent_boot.trn_boot import _ntff_profile_via_ctypes

        hook = _ntff_profile_via_ctypes("/opt/axon/libaxon_pjrt.so")
        if hook is not None:
            mod._hook = hook
    except Exception as e:  # degrade to untraced run
        print(f"NTFF hook setup failed ({e}); tracing will be skipped")


def kernel(x, Wk, Wq, Wv, _trace=False, _trace_kwargs=None):
    if _trace:
        _ensure_axon_ntff_hook()
    in_maps = _host_inputs(x, Wk, Wq, Wv)
    nc = _build_program()
    res = bass_utils.run_bass_kernel_spmd(
        nc, in_maps, list(range(B)), trace=_trace, **(_trace_kwargs or {})
    )
    out = np.stack(
        [_unpack_out(res.results[b]["out"]) for b in range(B)], axis=0
    )
    if _trace:
        kernel.last_results = res
    return out.astype(np.float32)
